# revision 28
# baseline (speedup 1.0000x reference)
"""Trainium2 kernel for nn_Attention (B=8, S=2048, D=768, H=12, DH=64, R=64).

Sharding: data-parallel over batch -> 1 batch element per NeuronCore (8 cores).
No collectives. LayerNorm affine folded into QKV weights on host.

v2 architecture (per core) — changes vs v1 baseline:
  * softmax exp split across TWO engines: even key-tiles on ScalarE (ACT
    table exp), odd key-tiles on the DVE via a custom-DVE op EXP_PS16_ANT
    (p(x)^16 with p = a x^2 + b x + c, one 8-stage uOp, ~0.7% rel err,
    softmax-invariant scale). Removes the ACT ceiling (1336 ns/tile).
  * ctx accumulation flipped: e_t [128keys, 128q] slices become the PE
    stationary, moving = [ones | v_head] [128, 65] -> out [128q, 65] PSUM
    (col 0 = softmax denominator). Full 128-wide PE output vs 65-wide in
    v1: ctx column count halves.
  * normalization: denominator now lives in the q PARTITION dim ->
    per-partition reciprocal + tensor_scalar multiply (DVE), no DRAM
    broadcast round-trips. Normalized ctx transposed back to feature-major
    via PE transpose (both heads of an hp share one [128,128] transpose);
    evacuated by ScalarE copies.
  * PSUM accumulator banks (acc[128,4,65] per head) are zeroed by ONE
    whole-bank start=True matmul (zero stationary); all ctx matmuls
    accumulate with start=False. Per-region start=True would corrupt
    sibling regions (PSUM zero regions are bank-granular).
  * ctx matmuls software-pipelined DEPTH=3 tiles behind scores/exp so the
    in-order PE queue never head-of-line blocks on the exp engines; the
    bank-zeroing matmuls are emitted at t==DEPTH for the same reason.
  * GpSimd (Pool) is compute-slow and cannot access PSUM on TRN2 - it
    only issues weight-DMA descriptors here.
"""

import sys

sys.path.insert(0, "/opt/trn_rl_repo")

import numpy as np

import concourse.bass as bass
import concourse.mybir as mybir
import concourse.tile as tile
from concourse import bacc, bass_utils
from concourse.masks import make_identity

F32 = mybir.dt.float32
BF16 = mybir.dt.bfloat16

B, S, D = 8, 2048, 768
H, DH, R = 12, 64, 64
EPS = 1e-6
NCORES = 8

ST = S // 128          # 16 s-tiles
DT = D // 128          # 6 d-tiles
HB = 66                # per-head block width in v (ones | v(64) | ones)
NC_CHUNK = 512         # q-positions per attention chunk
NCHUNK = S // NC_CHUNK # 4

# ---------------------------------------------------------------------------
# Custom DVE op: out = p(x)^16, p = A x^2 + B x + C  ~=  exp(x/8) on |x|<=28
# (softmax-grade approx; uniform scale ~0.999 folds out in softmax)
# ---------------------------------------------------------------------------
from concourse import dve_ops
from concourse.dve_ops import DveOp
from concourse.dve_spec import C0, C1, C2, Spec, Src0, lower, sq
from concourse.dve_uop import DveOpSpec

EXP_NAME = "EXP_PS16_ANT"
EXP_A = 3.0415196395345227e-05
EXP_B = 7.857364116999976e-03
EXP_C = 1.0000041215405195


def _ref_exp_ps16(in0, in1, s0, s1, imm2):
    x = in0.astype(np.float32)
    t = ((s0 * x + s1) * x + imm2).astype(np.float32)
    for _ in range(4):
        t = (t * t).astype(np.float32)
    return t


def _register_exp_op():
    if EXP_NAME in dve_ops._SUB_OPCODE_FOR_NAME:
        return next(op for op in dve_ops.OPS if op.name == EXP_NAME)
    body = sq(sq(sq(sq((Src0 * C0 + C1) * Src0 + C2))))
    spec = Spec(body=body, reference=_ref_exp_ps16)
    row = max(dve_ops._SUB_OPCODE_FOR_NAME.values()) + 1
    assert row < 0x20
    dve_ops._SUB_OPCODE_FOR_NAME[EXP_NAME] = row
    shas = {}
    for ver in ("v3", "v4"):
        try:
            s = DveOpSpec(name=EXP_NAME, opcode=row, uops=lower(spec, ver=ver),
                          rd1_en=False)
            shas[ver] = s.sha(ver)
        except Exception:
            pass
    op = DveOp(EXP_NAME, spec, subdim=False, uops_sha=shas)
    dve_ops.OPS.append(op)
    dve_ops.CUSTOM_DVE_SPECS[EXP_NAME] = spec
    return op


EXP_OP = _register_exp_op()


def build_nc() -> bass.Bass:
    nc = bacc.Bacc(None, target_bir_lowering=False, debug=False)

    y_ext = nc.declare_dram_parameter("y", [S, D], F32, isOutput=False)
    qT_kv_ext = nc.declare_dram_parameter("qkv_wT_kv", [D, 2 * D], BF16, isOutput=False)
    qT_q_ext = nc.declare_dram_parameter("qkv_wT_q", [D, D], BF16, isOutput=False)
    qkvb_ext = nc.declare_dram_parameter("qkv_b_eff", [3 * D], F32, isOutput=False)
    msaT_ext = nc.declare_dram_parameter("msa_wT", [D, D], BF16, isOutput=False)
    a1T_ext = nc.declare_dram_parameter("a1_wT", [D, R], BF16, isOutput=False)
    a1b_ext = nc.declare_dram_parameter("a1_b", [R], F32, isOutput=False)
    a2T_ext = nc.declare_dram_parameter("a2_wT_aug", [128, D], BF16, isOutput=False)
    out_ext = nc.declare_dram_parameter("out", [S, D], F32, isOutput=True)

    with tile.TileContext(nc) as tc:
        _build(tc, y_ext, qT_kv_ext, qT_q_ext, qkvb_ext, msaT_ext, a1T_ext,
               a1b_ext, a2T_ext, out_ext)
    nc.compile()
    return nc


def _build(tc, y_ext, qT_kv_ext, qT_q_ext, qkvb_ext, msaT_ext, a1T_ext,
           a1b_ext, a2T_ext, out_ext):
    from contextlib import ExitStack

    nc = tc.nc
    EXP = mybir.ActivationFunctionType.Exp
    RELU = mybir.ActivationFunctionType.Relu

    def dve_exp(out, in_):
        return nc.vector._custom_dve(
            EXP_OP, out=out, in0=in_, s0=EXP_A, s1=EXP_B, imm2=EXP_C
        )

    with ExitStack() as stack:
        ec = stack.enter_context
        # ---------------- long-lived pools ----------------
        consts = ec(tc.tile_pool(name="consts", bufs=1))
        big = ec(tc.tile_pool(name="big", bufs=1))

        ident = consts.tile([128, 128], BF16)
        make_identity(nc, ident)

        # weights go on the gpsimd DMA queue so the y tiles aren't stuck
        # behind them on the sync queue. kvwT first (needed at ~15us for the
        # first v-projection), bulky late-use weights (msaT, qwT) after.
        kvwT_sb = consts.tile([128, DT, 2 * D], BF16, tag="kvwT")
        nc.gpsimd.dma_start(
            out=kvwT_sb, in_=qT_kv_ext.ap().rearrange("(ko p) j -> p ko j", p=128)
        )
        msaT_sb_w = consts.tile([128, DT, D], BF16)
        nc.gpsimd.dma_start(
            out=msaT_sb_w, in_=msaT_ext.ap().rearrange("(ko p) j -> p ko j", p=128)
        )
        a1T_sb = consts.tile([128, DT, R], BF16)
        nc.gpsimd.dma_start(
            out=a1T_sb, in_=a1T_ext.ap().rearrange("(ko p) j -> p ko j", p=128)
        )
        a2T_sb = consts.tile([128, D], BF16)
        nc.gpsimd.dma_start(out=a2T_sb, in_=a2T_ext.ap())

        # q/k proj biases: [768] -> [128, 6] each
        qb_sb = consts.tile([128, DT], F32)
        nc.gpsimd.dma_start(
            out=qb_sb, in_=qkvb_ext.ap()[:D].rearrange("(jt p) -> p jt", p=128)
        )
        kb_sb = consts.tile([128, DT], F32)
        nc.gpsimd.dma_start(
            out=kb_sb, in_=qkvb_ext.ap()[D:2 * D].rearrange("(jt p) -> p jt", p=128)
        )
        # v bias broadcast across partitions: [768] -> [128, 768] (bf16)
        vb_src = qkvb_ext.ap()[2 * D:]
        vb_bcast = bass.AP(tensor=vb_src.tensor, offset=vb_src.offset,
                           ap=[[0, 128]] + list(vb_src.ap))
        vbias_sb = consts.tile([128, D], BF16)
        nc.gpsimd.dma_start(out=vbias_sb, in_=vb_bcast)

        a1b_sb = consts.tile([64, 1], F32)
        nc.gpsimd.dma_start(out=a1b_sb, in_=a1b_ext.ap()[:, None])

        eps_sb = consts.tile([128, 1], F32)
        nc.vector.memset(eps_sb, EPS)

        # zero stationary + dummy rhs for PSUM bank-zeroing matmuls (the
        # ctx accumulator banks hold 4 q-subtile regions; PSUM zero regions
        # are bank-granular, so each bank gets ONE whole-bank start=True
        # matmul and every ctx matmul accumulates with start=False)
        zw_sb = consts.tile([128, 128], BF16, tag="zw")
        nc.vector.memset(zw_sb, 0.0)
        zr_sb = consts.tile([128, 4 * 65], BF16, tag="zr")
        nc.vector.memset(zr_sb, 0.0)

        # q-projection weights live through the whole kernel (q deferred)
        qwT_sb = consts.tile([128, DT, D], BF16, tag="qwT")
        nc.gpsimd.dma_start(
            out=qwT_sb, in_=qT_q_ext.ap().rearrange("(ko p) j -> p ko j", p=128)
        )

        # ---------------- big activation tensors ----------------
        qT_sb = big.tile([128, DT, S], BF16, tag="qT")
        kT_sb = big.tile([128, DT, S], BF16, tag="kT")
        v_sb = big.tile([128, ST, H * HB], BF16, tag="v")
        xT_sb = big.tile([128, DT, S], BF16, tag="xT")

        v_blocks = v_sb.rearrange("p t (h u) -> p t h u", u=HB)
        nc.vector.memset(v_blocks[:, :, :, 0:1], 1.0)

        # ---------------- phase 1: LN + transpose + k/v (+q chunk 0) -------
        with tc.tile_pool(name="p1", bufs=5) as temps, \
             tc.tile_pool(name="p1small", bufs=8) as small, \
             tc.tile_pool(name="p1tr", bufs=2, space="PSUM") as psum_tr, \
             tc.tile_pool(name="p1mm", bufs=6, space="PSUM") as psum_p1:

            for sc in range(4):
                for st in range(4 * sc, 4 * sc + 4):
                    y_t = temps.tile([128, D], F32, tag="y")
                    nc.sync.dma_start(
                        out=y_t, in_=y_ext[st * 128:(st + 1) * 128, :])

                    stats = small.tile([128, 3, 6], F32, tag="stats")
                    y_grp = y_t.rearrange("p (g c) -> p g c", g=3)
                    for g in range(3):
                        nc.vector.bn_stats(out=stats[:, g, :], in_=y_grp[:, g, :])
                    mv = small.tile([128, 2], F32, tag="mv")
                    nc.vector.bn_aggr(out=mv, in_=stats)

                    rstd = small.tile([128, 1], F32, tag="rstd")
                    nc.scalar.activation(
                        out=rstd, in_=mv[:, 1:2],
                        func=mybir.ActivationFunctionType.Sqrt, bias=eps_sb,
                        scale=1.0,
                    )
                    nc.vector.reciprocal(out=rstd, in_=rstd)
                    x_bf = temps.tile([128, D], BF16, tag="xbf")
                    nc.vector.tensor_scalar(
                        out=x_bf, in0=y_t, scalar1=mv[:, 0:1], scalar2=rstd,
                        op0=mybir.AluOpType.subtract, op1=mybir.AluOpType.mult,
                    )
                    for dt in range(DT):
                        tr = psum_tr.tile([128, 128], BF16, tag="tr")
                        nc.tensor.transpose(
                            tr, x_bf[:, dt * 128:(dt + 1) * 128], ident)
                        if dt % 2 == 0:
                            nc.scalar.copy(
                                out=xT_sb[:, dt, st * 128:(st + 1) * 128],
                                in_=tr)
                        else:
                            nc.vector.tensor_copy(
                                out=xT_sb[:, dt, st * 128:(st + 1) * 128],
                                in_=tr)

                    # v projection for this s-tile (+bias), into 66-blocks
                    for jc, (j0, jw) in enumerate(((0, 512), (512, 256))):
                        vp = psum_p1.tile([128, 512], F32, tag="mm")
                        for kd in range(DT):
                            nc.tensor.matmul(
                                vp[:, :jw],
                                lhsT=xT_sb[:, kd, st * 128:(st + 1) * 128],
                                rhs=kvwT_sb[:, kd, D + j0: D + j0 + jw],
                                start=(kd == 0), stop=(kd == DT - 1),
                            )
                        h0 = j0 // 64
                        nh = jw // 64
                        nc.vector.tensor_add(
                            out=v_blocks[:, st, h0:h0 + nh, 1:65],
                            in0=vp[:, :jw].rearrange("p (h e) -> p h e", e=64),
                            in1=vbias_sb[:, j0:j0 + jw].rearrange(
                                "p (h e) -> p h e", e=64),
                        )

                # k projection for this s-chunk
                for jt in range(DT):
                    kp = psum_p1.tile([128, 512], F32, tag="mm")
                    for kd in range(DT):
                        nc.tensor.matmul(
                            kp,
                            lhsT=kvwT_sb[:, kd, jt * 128:(jt + 1) * 128],
                            rhs=xT_sb[:, kd, sc * 512:(sc + 1) * 512],
                            start=(kd == 0), stop=(kd == DT - 1),
                        )
                    nc.scalar.activation(
                        out=kT_sb[:, jt, sc * 512:(sc + 1) * 512], in_=kp,
                        func=mybir.ActivationFunctionType.Identity,
                        bias=kb_sb[:, jt:jt + 1], scale=1.0,
                    )

        # ---------------- attention + fused msa/adapter/output -------------
        with tc.tile_pool(name="sc", bufs=2, space="PSUM") as psum_sc, \
             tc.tile_pool(name="acc", bufs=2, space="PSUM") as psum_acc, \
             tc.tile_pool(name="mm", bufs=2, space="PSUM") as psum_mm, \
             tc.tile_pool(name="et", bufs=6) as e_pool, \
             tc.tile_pool(name="nrm", bufs=4) as nrm_pool, \
             tc.tile_pool(name="rec", bufs=4) as rec_pool, \
             tc.tile_pool(name="ctxp", bufs=2) as ctx_pool, \
             tc.tile_pool(name="msap", bufs=1) as msa_pool, \
             tc.tile_pool(name="outp", bufs=2) as out_pool:

            # per-chunk state carried between chunk iterations
            prev = {}

            def q_jt(cq, jt):
                """q projection for chunk cq, j-tile jt."""
                cs = cq * NC_CHUNK
                qp = psum_mm.tile([128, 512], F32, tag="mm")
                for kd in range(DT):
                    nc.tensor.matmul(
                        qp,
                        lhsT=qwT_sb[:, kd, jt * 128:(jt + 1) * 128],
                        rhs=xT_sb[:, kd, cs:cs + 512],
                        start=(kd == 0), stop=(kd == DT - 1),
                    )
                nc.scalar.activation(
                    out=qT_sb[:, jt, cs:cs + 512], in_=qp,
                    func=mybir.ActivationFunctionType.Identity,
                    bias=qb_sb[:, jt:jt + 1], scale=1.0,
                )

            def emit_attention_hp(c, hp, ctxT_c, fq=None):
                cs = c * NC_CHUNK
                hA, hB = 2 * hp, 2 * hp + 1
                accA = psum_acc.tile([128, 4, 65], F32, tag="acc")
                accB = psum_acc.tile([128, 4, 65], F32, tag="acc")
                # one whole-bank zeroing matmul per accumulator bank (see
                # zw_sb comment); all ctx matmuls then accumulate onto it.
                accA_flat = accA.rearrange("p a b -> p (a b)")
                accB_flat = accB.rearrange("p a b -> p (a b)")
                es = {}
                # software pipeline: ctx for tile t-DEPTH is emitted after
                # scores/exp of tile t so the in-order PE queue never
                # head-of-line blocks on the exp engines.
                DEPTH = 3
                for t in range(ST + DEPTH):
                    if t < ST:
                        s_t = psum_sc.tile([128, 1024], F32, tag="s")
                        nc.tensor.matmul(
                            s_t[:, 0:512],
                            lhsT=kT_sb[0:64, hp, t * 128:(t + 1) * 128],
                            rhs=qT_sb[0:64, hp, cs:cs + 512],
                            start=True, stop=True, tile_position=(0, 0),
                        )
                        nc.tensor.matmul(
                            s_t[:, 512:1024],
                            lhsT=kT_sb[64:128, hp, t * 128:(t + 1) * 128],
                            rhs=qT_sb[64:128, hp, cs:cs + 512],
                            start=True, stop=True, tile_position=(64, 0),
                        )
                        e_t = e_pool.tile([128, 1024], BF16, tag="et")
                        if t % 2 == 0:
                            nc.scalar.activation(
                                out=e_t, in_=s_t, func=EXP,
                                scale=float(1.0 / np.sqrt(DH)),
                            )
                        else:
                            dve_exp(e_t, s_t)
                        es[t] = e_t
                    if t >= DEPTH and fq and ((t - DEPTH) % 2 == 0):
                        # filler emitted BEFORE the ctx batch / zero-MMs:
                        # those are the exp-wait and acc-wait points, so the
                        # filler's duration becomes extra latency cover
                        fq.pop(0)()
                    if t == DEPTH:
                        # bank-zeroing matmuls emitted late so they don't
                        # head-of-line block the PE queue while the previous
                        # hp's normalize still reads the acc banks
                        nc.tensor.matmul(accA_flat, lhsT=zw_sb, rhs=zr_sb,
                                         start=True, stop=False)
                        nc.tensor.matmul(accB_flat, lhsT=zw_sb, rhs=zr_sb,
                                         start=True, stop=False)
                    if t >= DEPTH:
                        tc_ = t - DEPTH
                        e_c = es.pop(tc_)
                        vblk = v_sb[:, tc_, :]
                        vA = vblk[:, hA * HB: hA * HB + 65]
                        vB = vblk[:, hB * HB: hB * HB + 65]
                        last = (tc_ == ST - 1)
                        for qs in range(4):
                            nc.tensor.matmul(
                                accA[:, qs, :],
                                lhsT=e_c[:, qs * 128:(qs + 1) * 128],
                                rhs=vA,
                                start=False, stop=(last and qs == 3),
                            )
                        for qs in range(4):
                            nc.tensor.matmul(
                                accB[:, qs, :],
                                lhsT=e_c[:, 512 + qs * 128:512 + (qs + 1) * 128],
                                rhs=vB,
                                start=False, stop=(last and qs == 3),
                            )
                return accA, accB

            def emit_normalize_hp(c, hp, ctxT_c, accA, accB):
                """1/denominator lives in the q partition dim: batched
                reciprocal + per-partition tensor_scalar mult (GpSimd),
                then one [128,128] PE transpose per q-subtile covers both
                heads (they fill complementary 64-part halves of ctxT)."""
                recA = rec_pool.tile([128, 4, 1], F32, tag="rec")
                recB = rec_pool.tile([128, 4, 1], F32, tag="rec")
                nc.vector.reciprocal(out=recA, in_=accA[:, :, 0:1])
                nc.vector.reciprocal(out=recB, in_=accB[:, :, 0:1])
                for qs in range(4):
                    nrm = nrm_pool.tile([128, 128], BF16, tag="nrm")
                    nc.vector.tensor_scalar_mul(
                        out=nrm[:, 0:64], in0=accA[:, qs, 1:65],
                        scalar1=recA[:, qs, :],
                    )
                    nc.vector.tensor_scalar_mul(
                        out=nrm[:, 64:128], in0=accB[:, qs, 1:65],
                        scalar1=recB[:, qs, :],
                    )
                    tr = psum_mm.tile([128, 512], BF16, tag="mm")
                    nc.tensor.transpose(tr[:, 0:128], nrm, ident)
                    nc.scalar.copy(
                        out=ctxT_c[:, hp, qs * 128:(qs + 1) * 128],
                        in_=tr[:, 0:128],
                    )

            def filler_adapter(c, ctxT, hT, pool, ptag="mm"):
                """h = relu(ctx @ a1_eff.T + b) straight from ctxT
                (msa_w folded into a1 on the host)."""
                hp_ps = pool.tile([128, 512], F32, tag=ptag)
                for kd in range(DT):
                    nc.tensor.matmul(
                        hp_ps[0:64, :],
                        lhsT=a1T_sb[:, kd, :],
                        rhs=ctxT[:, kd, :],
                        start=(kd == 0), stop=(kd == DT - 1),
                    )
                nc.scalar.activation(
                    out=hT[0:64, :], in_=hp_ps[0:64, :], func=RELU,
                    bias=a1b_sb, scale=1.0,
                )

            def out_st_part(cell, sti, ctxT, hT, pool, j0, jw, ptag="mm"):
                """one e-slice of msa+adapter for output s-tile sti; the
                second part (j0=512) issues the DMA."""
                ql = slice(sti * 128, (sti + 1) * 128)
                if cell.get("o_t") is None:
                    o_t_new = out_pool.tile([128, D], F32, tag="out")
                    cell["o_t"] = o_t_new
                o_t = cell["o_t"]
                op = pool.tile([128, 512], F32, tag=ptag)
                for kd in range(DT):
                    nc.tensor.matmul(
                        op[:, :jw],
                        lhsT=ctxT[:, kd, ql],
                        rhs=msaT_sb_w[:, kd, j0:j0 + jw],
                        start=(kd == 0), stop=False,
                    )
                nc.tensor.matmul(
                    op[:, :jw],
                    lhsT=hT[:, ql],
                    rhs=a2T_sb[:, j0:j0 + jw],
                    start=False, stop=True,
                )
                if j0 == 0:
                    nc.vector.tensor_copy(out=o_t[:, :jw], in_=op[:, :jw])
                else:
                    nc.scalar.copy(out=o_t[:, j0:j0 + jw], in_=op[:, :jw])
                if j0 == 512:
                    st = cell["st"]
                    nc.sync.dma_start(
                        out=out_ext[st * 128:(st + 1) * 128, :], in_=o_t)

            def filler_out_st(c, sti, ctxT, hT, pool, ptag="mm"):
                cell = {"o_t": None, "st": 4 * c + sti}
                out_st_part(cell, sti, ctxT, hT, pool, 0, 512, ptag)
                out_st_part(cell, sti, ctxT, hT, pool, 512, 256, ptag)

            def out_st_closures(c, sti, ctxT, hT):
                cell = {"o_t": None, "st": 4 * c + sti}
                return [
                    lambda: out_st_part(cell, sti, ctxT, hT, psum_mm, 0, 512),
                    lambda: out_st_part(cell, sti, ctxT, hT, psum_mm, 512, 256),
                ]

            def emit_output_phase_slices(c_prev, ctxT, hT):
                """Fine-grained filler closures for chunk c_prev's
                adapter+output, interleaved into the next chunk's t-loops."""
                # NOTE: emission order is program order — the adapter (which
                # writes hT) MUST be emitted before any out-st work reads hT.
                slices = [[] for _ in range(6)]
                slices[0].append(lambda: filler_adapter(c_prev, ctxT, hT, psum_mm))
                for i in range(4):
                    slices[i + 1].extend(out_st_closures(c_prev, i, ctxT, hT))
                return slices

            for c in range(NCHUNK):
                ctxT_c = ctx_pool.tile([128, DT, 512], BF16, tag="ctxT")

                # build filler slices from previous chunk
                slices = [[] for _ in range(6)]
                if c > 0:
                    hT = msa_pool.tile([128, 512], BF16, tag="hT")
                    nc.vector.memset(hT[64:128, :], 0.0)
                    nc.vector.memset(hT[64:65, :], 1.0)
                    ms = emit_output_phase_slices(c - 1, prev["ctxT"], hT)
                    for i in range(6):
                        slices[i].extend(ms[i])
                if c < NCHUNK - 1:
                    # q projection for chunk c+1: one jt closure per slot
                    for i in range(6):
                        slices[i].append(lambda jt=i, cq=c + 1: q_jt(cq, jt))

                for hp in range(DT):
                    if c == 0 and hp == 0:
                        q_jt(0, 0)
                    fq = list(slices[hp])
                    if c == 0 and hp + 1 < DT:
                        # next head-pair's chunk-0 q projection as the first
                        # filler inside this hp's t-loop (not a serial block)
                        fq.insert(0, lambda h=hp + 1: q_jt(0, h))
                    accA, accB = emit_attention_hp(c, hp, ctxT_c, fq)
                    emit_normalize_hp(c, hp, ctxT_c, accA, accB)
                    for fn in fq:
                        fn()
                prev["ctxT"] = ctxT_c

            # tail: adapter/output for the last chunk (msa folded into the
            # out accumulation; adapter straight from ctxT)
            ctxT3 = prev["ctxT"]
            hT = msa_pool.tile([128, 512], BF16, tag="hT")
            nc.vector.memset(hT[64:128, :], 0.0)
            nc.vector.memset(hT[64:65, :], 1.0)
            filler_adapter(NCHUNK - 1, ctxT3, hT, psum_mm)
            for sti in range(4):
                pool, tag = (psum_sc, "s") if sti % 2 == 0 else (psum_mm, "mm")
                filler_out_st(NCHUNK - 1, sti, ctxT3, hT, pool, tag)


_NC_CACHE = None


def _get_nc():
    global _NC_CACHE
    if _NC_CACHE is None:
        _NC_CACHE = build_nc()
    return _NC_CACHE


def _prep_in_maps(y, ln_g, ln_b, qkv_w, qkv_b, msa_w, a1_w, a1_b, a2_w, a2_b):
    f = np.float32
    y = np.asarray(y, f)
    ln_g = np.asarray(ln_g, f)
    ln_b = np.asarray(ln_b, f)
    qkv_w = np.asarray(qkv_w, f)
    qkv_b = np.asarray(qkv_b, f)
    msa_w = np.asarray(msa_w, f)
    a1_w = np.asarray(a1_w, f)
    a1_b = np.asarray(a1_b, f)
    a2_w = np.asarray(a2_w, f)
    a2_b = np.asarray(a2_b, f)

    import ml_dtypes
    bf = ml_dtypes.bfloat16

    # Fold LN affine into QKV: (g*xn + b) @ W.T + c == xn @ (W*g).T + (W@b + c)
    qkv_wT = np.ascontiguousarray((qkv_w * ln_g[None, :]).T)          # [768, 2304]
    qkv_b_eff = (qkv_b + qkv_w @ ln_b).astype(f)                      # [2304]
    qkv_wT_q = np.ascontiguousarray(qkv_wT[:, :D]).astype(bf)
    qkv_wT_kv = np.ascontiguousarray(qkv_wT[:, D:]).astype(bf)
    msa_wT = np.ascontiguousarray(msa_w.T).astype(bf)                 # [768, 768]
    # fold msa into adapter layer 1: relu(msa@a1.T) == relu(ctx@(a1@msa_w).T)
    a1_wT = np.ascontiguousarray((a1_w @ msa_w).T).astype(bf)         # [768, 64]
    a2_aug = np.zeros((128, D), f)                                    # [128, 768]
    a2_aug[:R] = a2_w.T
    a2_aug[R] = a2_b
    a2_aug = a2_aug.astype(bf)

    shared = {
        "qkv_wT_q": qkv_wT_q, "qkv_wT_kv": qkv_wT_kv, "qkv_b_eff": qkv_b_eff,
        "msa_wT": msa_wT, "a1_wT": a1_wT, "a1_b": a1_b, "a2_wT_aug": a2_aug,
    }
    in_maps = [dict(shared, y=np.ascontiguousarray(y[b])) for b in range(NCORES)]
    return in_maps


def run(trace=False, **inputs):
    in_maps = _prep_in_maps(**inputs)
    nc = _get_nc()
    res = bass_utils.run_bass_kernel_spmd(
        nc, in_maps, core_ids=list(range(NCORES)), trace=trace
    )
    out = np.stack([r["out"] for r in res.results], axis=0)
    return out.astype(np.float32), res


def kernel(**inputs) -> np.ndarray:
    out, _ = run(trace=False, **inputs)
    return out


# revision 29
# speedup vs baseline: 1.0156x; 1.0156x over previous
"""Trainium2 kernel for nn_Attention (B=8, S=2048, D=768, H=12, DH=64, R=64).

Sharding: data-parallel over batch -> 1 batch element per NeuronCore (8 cores).
No collectives. LayerNorm affine folded into QKV weights on host.

v2 architecture (per core) — changes vs v1 baseline:
  * softmax exp split across TWO engines: even key-tiles on ScalarE (ACT
    table exp), odd key-tiles on the DVE via a custom-DVE op EXP_PS16_ANT
    (p(x)^16 with p = a x^2 + b x + c, one 8-stage uOp, ~0.7% rel err,
    softmax-invariant scale). Removes the ACT ceiling (1336 ns/tile).
  * ctx accumulation flipped: e_t [128keys, 128q] slices become the PE
    stationary, moving = [ones | v_head] [128, 65] -> out [128q, 65] PSUM
    (col 0 = softmax denominator). Full 128-wide PE output vs 65-wide in
    v1: ctx column count halves.
  * normalization: denominator now lives in the q PARTITION dim ->
    per-partition reciprocal + tensor_scalar multiply (DVE), no DRAM
    broadcast round-trips. Normalized ctx transposed back to feature-major
    via PE transpose (both heads of an hp share one [128,128] transpose);
    evacuated by ScalarE copies.
  * PSUM accumulator banks (acc[128,4,65] per head) are zeroed by ONE
    whole-bank start=True matmul (zero stationary); all ctx matmuls
    accumulate with start=False. Per-region start=True would corrupt
    sibling regions (PSUM zero regions are bank-granular).
  * ctx matmuls software-pipelined DEPTH=3 tiles behind scores/exp so the
    in-order PE queue never head-of-line blocks on the exp engines; the
    bank-zeroing matmuls are emitted at t==DEPTH for the same reason.
  * GpSimd (Pool) is compute-slow and cannot access PSUM on TRN2 - it
    only issues weight-DMA descriptors here.
"""

import sys

sys.path.insert(0, "/opt/trn_rl_repo")

import numpy as np

import concourse.bass as bass
import concourse.mybir as mybir
import concourse.tile as tile
from concourse import bacc, bass_utils
from concourse.masks import make_identity

F32 = mybir.dt.float32
BF16 = mybir.dt.bfloat16

B, S, D = 8, 2048, 768
H, DH, R = 12, 64, 64
EPS = 1e-6
NCORES = 8

ST = S // 128          # 16 s-tiles
DT = D // 128          # 6 d-tiles
HB = 66                # per-head block width in v (ones | v(64) | ones)
NC_CHUNK = 512         # q-positions per attention chunk
NCHUNK = S // NC_CHUNK # 4

# ---------------------------------------------------------------------------
# Custom DVE op: out = p(x)^16, p = A x^2 + B x + C  ~=  exp(x/8) on |x|<=28
# (softmax-grade approx; uniform scale ~0.999 folds out in softmax)
# ---------------------------------------------------------------------------
from concourse import dve_ops
from concourse.dve_ops import DveOp
from concourse.dve_spec import C0, C1, C2, Spec, Src0, lower, sq
from concourse.dve_uop import DveOpSpec

EXP_NAME = "EXP_PS16_ANT"
EXP_A = 3.0415196395345227e-05
EXP_B = 7.857364116999976e-03
EXP_C = 1.0000041215405195


def _ref_exp_ps16(in0, in1, s0, s1, imm2):
    x = in0.astype(np.float32)
    t = ((s0 * x + s1) * x + imm2).astype(np.float32)
    for _ in range(4):
        t = (t * t).astype(np.float32)
    return t


def _register_exp_op():
    if EXP_NAME in dve_ops._SUB_OPCODE_FOR_NAME:
        return next(op for op in dve_ops.OPS if op.name == EXP_NAME)
    body = sq(sq(sq(sq((Src0 * C0 + C1) * Src0 + C2))))
    spec = Spec(body=body, reference=_ref_exp_ps16)
    row = max(dve_ops._SUB_OPCODE_FOR_NAME.values()) + 1
    assert row < 0x20
    dve_ops._SUB_OPCODE_FOR_NAME[EXP_NAME] = row
    shas = {}
    for ver in ("v3", "v4"):
        try:
            s = DveOpSpec(name=EXP_NAME, opcode=row, uops=lower(spec, ver=ver),
                          rd1_en=False)
            shas[ver] = s.sha(ver)
        except Exception:
            pass
    op = DveOp(EXP_NAME, spec, subdim=False, uops_sha=shas)
    dve_ops.OPS.append(op)
    dve_ops.CUSTOM_DVE_SPECS[EXP_NAME] = spec
    return op


EXP_OP = _register_exp_op()


def build_nc() -> bass.Bass:
    nc = bacc.Bacc(None, target_bir_lowering=False, debug=False)

    y_ext = nc.declare_dram_parameter("y", [S, D], F32, isOutput=False)
    qT_kv_ext = nc.declare_dram_parameter("qkv_wT_kv", [D, 2 * D], BF16, isOutput=False)
    qT_q_ext = nc.declare_dram_parameter("qkv_wT_q", [D, D], BF16, isOutput=False)
    qkvb_ext = nc.declare_dram_parameter("qkv_b_eff", [3 * D], F32, isOutput=False)
    msaT_ext = nc.declare_dram_parameter("msa_wT", [D, D], BF16, isOutput=False)
    a1T_ext = nc.declare_dram_parameter("a1_wT", [D, R], BF16, isOutput=False)
    a1b_ext = nc.declare_dram_parameter("a1_b", [R], F32, isOutput=False)
    a2T_ext = nc.declare_dram_parameter("a2_wT_aug", [128, D], BF16, isOutput=False)
    out_ext = nc.declare_dram_parameter("out", [S, D], F32, isOutput=True)

    with tile.TileContext(nc) as tc:
        _build(tc, y_ext, qT_kv_ext, qT_q_ext, qkvb_ext, msaT_ext, a1T_ext,
               a1b_ext, a2T_ext, out_ext)
    nc.compile()
    return nc


def _build(tc, y_ext, qT_kv_ext, qT_q_ext, qkvb_ext, msaT_ext, a1T_ext,
           a1b_ext, a2T_ext, out_ext):
    from contextlib import ExitStack

    nc = tc.nc
    EXP = mybir.ActivationFunctionType.Exp
    RELU = mybir.ActivationFunctionType.Relu

    def dve_exp(out, in_):
        return nc.vector._custom_dve(
            EXP_OP, out=out, in0=in_, s0=EXP_A, s1=EXP_B, imm2=EXP_C
        )

    with ExitStack() as stack:
        ec = stack.enter_context
        # ---------------- long-lived pools ----------------
        consts = ec(tc.tile_pool(name="consts", bufs=1))
        big = ec(tc.tile_pool(name="big", bufs=1))

        ident = consts.tile([128, 128], BF16)
        make_identity(nc, ident)

        # weights go on the gpsimd DMA queue so the y tiles aren't stuck
        # behind them on the sync queue. kvwT first (needed at ~15us for the
        # first v-projection), bulky late-use weights (msaT, qwT) after.
        kvwT_sb = consts.tile([128, DT, 2 * D], BF16, tag="kvwT")
        nc.gpsimd.dma_start(
            out=kvwT_sb, in_=qT_kv_ext.ap().rearrange("(ko p) j -> p ko j", p=128)
        )
        msaT_sb_w = consts.tile([128, DT, D], BF16)
        nc.gpsimd.dma_start(
            out=msaT_sb_w, in_=msaT_ext.ap().rearrange("(ko p) j -> p ko j", p=128)
        )
        a1T_sb = consts.tile([128, DT, R], BF16)
        nc.gpsimd.dma_start(
            out=a1T_sb, in_=a1T_ext.ap().rearrange("(ko p) j -> p ko j", p=128)
        )
        a2T_sb = consts.tile([128, D], BF16)
        nc.gpsimd.dma_start(out=a2T_sb, in_=a2T_ext.ap())

        # q/k proj biases: [768] -> [128, 6] each
        qb_sb = consts.tile([128, DT], F32)
        nc.gpsimd.dma_start(
            out=qb_sb, in_=qkvb_ext.ap()[:D].rearrange("(jt p) -> p jt", p=128)
        )
        kb_sb = consts.tile([128, DT], F32)
        nc.gpsimd.dma_start(
            out=kb_sb, in_=qkvb_ext.ap()[D:2 * D].rearrange("(jt p) -> p jt", p=128)
        )
        # v bias broadcast across partitions: [768] -> [128, 768] (bf16)
        vb_src = qkvb_ext.ap()[2 * D:]
        vb_bcast = bass.AP(tensor=vb_src.tensor, offset=vb_src.offset,
                           ap=[[0, 128]] + list(vb_src.ap))
        vbias_sb = consts.tile([128, D], BF16)
        nc.gpsimd.dma_start(out=vbias_sb, in_=vb_bcast)

        a1b_sb = consts.tile([64, 1], F32)
        nc.gpsimd.dma_start(out=a1b_sb, in_=a1b_ext.ap()[:, None])

        eps_sb = consts.tile([128, 1], F32)
        nc.vector.memset(eps_sb, EPS)

        # zero stationary + dummy rhs for PSUM bank-zeroing matmuls (the
        # ctx accumulator banks hold 4 q-subtile regions; PSUM zero regions
        # are bank-granular, so each bank gets ONE whole-bank start=True
        # matmul and every ctx matmul accumulates with start=False)
        zw_sb = consts.tile([128, 128], BF16, tag="zw")
        nc.vector.memset(zw_sb, 0.0)
        zr_sb = consts.tile([128, 4 * 65], BF16, tag="zr")
        nc.vector.memset(zr_sb, 0.0)

        # q-projection weights live through the whole kernel (q deferred)
        qwT_sb = consts.tile([128, DT, D], BF16, tag="qwT")
        nc.gpsimd.dma_start(
            out=qwT_sb, in_=qT_q_ext.ap().rearrange("(ko p) j -> p ko j", p=128)
        )

        # ---------------- big activation tensors ----------------
        qT_sb = big.tile([128, DT, S], BF16, tag="qT")
        kT_sb = big.tile([128, DT, S], BF16, tag="kT")
        v_sb = big.tile([128, ST, H * HB], BF16, tag="v")
        xT_sb = big.tile([128, DT, S], BF16, tag="xT")

        v_blocks = v_sb.rearrange("p t (h u) -> p t h u", u=HB)
        nc.vector.memset(v_blocks[:, :, :, 0:1], 1.0)

        # ---------------- phase 1: LN + transpose + k/v (+q chunk 0) -------
        with tc.tile_pool(name="p1", bufs=5) as temps, \
             tc.tile_pool(name="p1small", bufs=8) as small, \
             tc.tile_pool(name="p1tr", bufs=2, space="PSUM") as psum_tr, \
             tc.tile_pool(name="p1mm", bufs=6, space="PSUM") as psum_p1:

            for sc in range(4):
                for st in range(4 * sc, 4 * sc + 4):
                    y_t = temps.tile([128, D], F32, tag="y")
                    nc.sync.dma_start(
                        out=y_t, in_=y_ext[st * 128:(st + 1) * 128, :])

                    stats = small.tile([128, 3, 6], F32, tag="stats")
                    y_grp = y_t.rearrange("p (g c) -> p g c", g=3)
                    for g in range(3):
                        nc.vector.bn_stats(out=stats[:, g, :], in_=y_grp[:, g, :])
                    mv = small.tile([128, 2], F32, tag="mv")
                    nc.vector.bn_aggr(out=mv, in_=stats)

                    rstd = small.tile([128, 1], F32, tag="rstd")
                    nc.scalar.activation(
                        out=rstd, in_=mv[:, 1:2],
                        func=mybir.ActivationFunctionType.Sqrt, bias=eps_sb,
                        scale=1.0,
                    )
                    nc.vector.reciprocal(out=rstd, in_=rstd)
                    x_bf = temps.tile([128, D], BF16, tag="xbf")
                    nc.vector.tensor_scalar(
                        out=x_bf, in0=y_t, scalar1=mv[:, 0:1], scalar2=rstd,
                        op0=mybir.AluOpType.subtract, op1=mybir.AluOpType.mult,
                    )
                    for dt in range(DT):
                        tr = psum_tr.tile([128, 128], BF16, tag="tr")
                        nc.tensor.transpose(
                            tr, x_bf[:, dt * 128:(dt + 1) * 128], ident)
                        if dt % 2 == 0:
                            nc.scalar.copy(
                                out=xT_sb[:, dt, st * 128:(st + 1) * 128],
                                in_=tr)
                        else:
                            nc.vector.tensor_copy(
                                out=xT_sb[:, dt, st * 128:(st + 1) * 128],
                                in_=tr)

                    # v projection for this s-tile (+bias), into 66-blocks
                    for jc, (j0, jw) in enumerate(((0, 512), (512, 256))):
                        vp = psum_p1.tile([128, 512], F32, tag="mm")
                        for kd in range(DT):
                            nc.tensor.matmul(
                                vp[:, :jw],
                                lhsT=xT_sb[:, kd, st * 128:(st + 1) * 128],
                                rhs=kvwT_sb[:, kd, D + j0: D + j0 + jw],
                                start=(kd == 0), stop=(kd == DT - 1),
                            )
                        h0 = j0 // 64
                        nh = jw // 64
                        nc.vector.tensor_add(
                            out=v_blocks[:, st, h0:h0 + nh, 1:65],
                            in0=vp[:, :jw].rearrange("p (h e) -> p h e", e=64),
                            in1=vbias_sb[:, j0:j0 + jw].rearrange(
                                "p (h e) -> p h e", e=64),
                        )

                # k projection for this s-chunk
                for jt in range(DT):
                    kp = psum_p1.tile([128, 512], F32, tag="mm")
                    for kd in range(DT):
                        nc.tensor.matmul(
                            kp,
                            lhsT=kvwT_sb[:, kd, jt * 128:(jt + 1) * 128],
                            rhs=xT_sb[:, kd, sc * 512:(sc + 1) * 512],
                            start=(kd == 0), stop=(kd == DT - 1),
                        )
                    nc.scalar.activation(
                        out=kT_sb[:, jt, sc * 512:(sc + 1) * 512], in_=kp,
                        func=mybir.ActivationFunctionType.Identity,
                        bias=kb_sb[:, jt:jt + 1], scale=1.0,
                    )

        # ---------------- attention + fused msa/adapter/output -------------
        with tc.tile_pool(name="sc", bufs=2, space="PSUM") as psum_sc, \
             tc.tile_pool(name="acc", bufs=2, space="PSUM") as psum_acc, \
             tc.tile_pool(name="mm", bufs=2, space="PSUM") as psum_mm, \
             tc.tile_pool(name="et", bufs=6) as e_pool, \
             tc.tile_pool(name="nrm", bufs=4) as nrm_pool, \
             tc.tile_pool(name="rec", bufs=4) as rec_pool, \
             tc.tile_pool(name="ctxp", bufs=2) as ctx_pool, \
             tc.tile_pool(name="msap", bufs=1) as msa_pool, \
             tc.tile_pool(name="outp", bufs=2) as out_pool:

            # per-chunk state carried between chunk iterations
            prev = {}

            def q_jt(cq, jt):
                """q projection for chunk cq, j-tile jt."""
                cs = cq * NC_CHUNK
                qp = psum_mm.tile([128, 512], F32, tag="mm")
                for kd in range(DT):
                    nc.tensor.matmul(
                        qp,
                        lhsT=qwT_sb[:, kd, jt * 128:(jt + 1) * 128],
                        rhs=xT_sb[:, kd, cs:cs + 512],
                        start=(kd == 0), stop=(kd == DT - 1),
                    )
                nc.scalar.activation(
                    out=qT_sb[:, jt, cs:cs + 512], in_=qp,
                    func=mybir.ActivationFunctionType.Identity,
                    bias=qb_sb[:, jt:jt + 1], scale=1.0,
                )

            def emit_attention_hp(c, hp, ctxT_c, fq=None):
                cs = c * NC_CHUNK
                hA, hB = 2 * hp, 2 * hp + 1
                accA = psum_acc.tile([128, 4, 65], F32, tag="acc")
                accB = psum_acc.tile([128, 4, 65], F32, tag="acc")
                # one whole-bank zeroing matmul per accumulator bank (see
                # zw_sb comment); all ctx matmuls then accumulate onto it.
                accA_flat = accA.rearrange("p a b -> p (a b)")
                accB_flat = accB.rearrange("p a b -> p (a b)")
                es = {}
                # software pipeline: ctx for tile t-DEPTH is emitted after
                # scores/exp of tile t so the in-order PE queue never
                # head-of-line blocks on the exp engines.
                DEPTH = 3
                for t in range(ST + DEPTH):
                    if t < ST:
                        s_t = psum_sc.tile([128, 1024], F32, tag="s")
                        nc.tensor.matmul(
                            s_t[:, 0:512],
                            lhsT=kT_sb[0:64, hp, t * 128:(t + 1) * 128],
                            rhs=qT_sb[0:64, hp, cs:cs + 512],
                            start=True, stop=True, tile_position=(0, 0),
                        )
                        nc.tensor.matmul(
                            s_t[:, 512:1024],
                            lhsT=kT_sb[64:128, hp, t * 128:(t + 1) * 128],
                            rhs=qT_sb[64:128, hp, cs:cs + 512],
                            start=True, stop=True, tile_position=(64, 0),
                        )
                        e_t = e_pool.tile([128, 1024], BF16, tag="et")
                        if t % 2 == 0:
                            nc.scalar.activation(
                                out=e_t, in_=s_t, func=EXP,
                                scale=float(1.0 / np.sqrt(DH)),
                            )
                        else:
                            dve_exp(e_t, s_t)
                        es[t] = e_t
                    if t == DEPTH:
                        # bank-zeroing matmuls emitted late so they don't
                        # head-of-line block the PE queue while the previous
                        # hp's normalize still reads the acc banks
                        nc.tensor.matmul(accA_flat, lhsT=zw_sb, rhs=zr_sb,
                                         start=True, stop=False)
                        nc.tensor.matmul(accB_flat, lhsT=zw_sb, rhs=zr_sb,
                                         start=True, stop=False)
                    if t >= DEPTH:
                        tc_ = t - DEPTH
                        e_c = es.pop(tc_)
                        vblk = v_sb[:, tc_, :]
                        vA = vblk[:, hA * HB: hA * HB + 65]
                        vB = vblk[:, hB * HB: hB * HB + 65]
                        last = (tc_ == ST - 1)
                        for qs in range(4):
                            nc.tensor.matmul(
                                accA[:, qs, :],
                                lhsT=e_c[:, qs * 128:(qs + 1) * 128],
                                rhs=vA,
                                start=False, stop=(last and qs == 3),
                            )
                        for qs in range(4):
                            nc.tensor.matmul(
                                accB[:, qs, :],
                                lhsT=e_c[:, 512 + qs * 128:512 + (qs + 1) * 128],
                                rhs=vB,
                                start=False, stop=(last and qs == 3),
                            )
                        # fine-grained fillers between ctx batches keep PE
                        # fed during exp waits without starving the exp
                        # engines behind a multi-us filler block
                        if fq and ((t - DEPTH) % 2 == 1):
                            fq.pop(0)()
                return accA, accB

            def emit_normalize_hp(c, hp, ctxT_c, accA, accB):
                """1/denominator lives in the q partition dim: batched
                reciprocal + per-partition tensor_scalar mult (GpSimd),
                then one [128,128] PE transpose per q-subtile covers both
                heads (they fill complementary 64-part halves of ctxT)."""
                recA = rec_pool.tile([128, 4, 1], F32, tag="rec")
                recB = rec_pool.tile([128, 4, 1], F32, tag="rec")
                nc.vector.reciprocal(out=recA, in_=accA[:, :, 0:1])
                nc.vector.reciprocal(out=recB, in_=accB[:, :, 0:1])
                for qs in range(4):
                    nrm = nrm_pool.tile([128, 128], BF16, tag="nrm")
                    nc.vector.tensor_scalar_mul(
                        out=nrm[:, 0:64], in0=accA[:, qs, 1:65],
                        scalar1=recA[:, qs, :],
                    )
                    nc.vector.tensor_scalar_mul(
                        out=nrm[:, 64:128], in0=accB[:, qs, 1:65],
                        scalar1=recB[:, qs, :],
                    )
                    tr = psum_mm.tile([128, 512], BF16, tag="mm")
                    nc.tensor.transpose(tr[:, 0:128], nrm, ident)
                    nc.scalar.copy(
                        out=ctxT_c[:, hp, qs * 128:(qs + 1) * 128],
                        in_=tr[:, 0:128],
                    )

            def filler_adapter(c, ctxT, hT, pool, ptag="mm"):
                """h = relu(ctx @ a1_eff.T + b) straight from ctxT
                (msa_w folded into a1 on the host)."""
                hp_ps = pool.tile([128, 512], F32, tag=ptag)
                for kd in range(DT):
                    nc.tensor.matmul(
                        hp_ps[0:64, :],
                        lhsT=a1T_sb[:, kd, :],
                        rhs=ctxT[:, kd, :],
                        start=(kd == 0), stop=(kd == DT - 1),
                    )
                nc.scalar.activation(
                    out=hT[0:64, :], in_=hp_ps[0:64, :], func=RELU,
                    bias=a1b_sb, scale=1.0,
                )

            def out_st_part(cell, sti, ctxT, hT, pool, j0, jw, ptag="mm"):
                """one e-slice of msa+adapter for output s-tile sti; the
                second part (j0=512) issues the DMA."""
                ql = slice(sti * 128, (sti + 1) * 128)
                if cell.get("o_t") is None:
                    o_t_new = out_pool.tile([128, D], F32, tag="out")
                    cell["o_t"] = o_t_new
                o_t = cell["o_t"]
                op = pool.tile([128, 512], F32, tag=ptag)
                for kd in range(DT):
                    nc.tensor.matmul(
                        op[:, :jw],
                        lhsT=ctxT[:, kd, ql],
                        rhs=msaT_sb_w[:, kd, j0:j0 + jw],
                        start=(kd == 0), stop=False,
                    )
                nc.tensor.matmul(
                    op[:, :jw],
                    lhsT=hT[:, ql],
                    rhs=a2T_sb[:, j0:j0 + jw],
                    start=False, stop=True,
                )
                if j0 == 0:
                    nc.vector.tensor_copy(out=o_t[:, :jw], in_=op[:, :jw])
                else:
                    nc.scalar.copy(out=o_t[:, j0:j0 + jw], in_=op[:, :jw])
                if j0 == 512:
                    st = cell["st"]
                    nc.sync.dma_start(
                        out=out_ext[st * 128:(st + 1) * 128, :], in_=o_t)

            def filler_out_st(c, sti, ctxT, hT, pool, ptag="mm"):
                cell = {"o_t": None, "st": 4 * c + sti}
                out_st_part(cell, sti, ctxT, hT, pool, 0, 512, ptag)
                out_st_part(cell, sti, ctxT, hT, pool, 512, 256, ptag)

            def out_st_closures(c, sti, ctxT, hT):
                cell = {"o_t": None, "st": 4 * c + sti}
                return [
                    lambda: out_st_part(cell, sti, ctxT, hT, psum_mm, 0, 512),
                    lambda: out_st_part(cell, sti, ctxT, hT, psum_mm, 512, 256),
                ]

            def emit_output_phase_slices(c_prev, ctxT, hT):
                """Fine-grained filler closures for chunk c_prev's
                adapter+output, interleaved into the next chunk's t-loops."""
                # NOTE: emission order is program order — the adapter (which
                # writes hT) MUST be emitted before any out-st work reads hT.
                slices = [[] for _ in range(6)]
                slices[0].append(lambda: filler_adapter(c_prev, ctxT, hT, psum_mm))
                for i in range(4):
                    slices[i + 1].extend(out_st_closures(c_prev, i, ctxT, hT))
                return slices

            for c in range(NCHUNK):
                ctxT_c = ctx_pool.tile([128, DT, 512], BF16, tag="ctxT")

                # build filler slices from previous chunk
                slices = [[] for _ in range(6)]
                if c > 0:
                    hT = msa_pool.tile([128, 512], BF16, tag="hT")
                    nc.vector.memset(hT[64:128, :], 0.0)
                    nc.vector.memset(hT[64:65, :], 1.0)
                    ms = emit_output_phase_slices(c - 1, prev["ctxT"], hT)
                    for i in range(6):
                        slices[i].extend(ms[i])
                if c < NCHUNK - 1:
                    # q projection for chunk c+1: one jt closure per slot
                    for i in range(6):
                        slices[i].append(lambda jt=i, cq=c + 1: q_jt(cq, jt))

                for hp in range(DT):
                    if c == 0 and hp == 0:
                        q_jt(0, 0)
                    fq = list(slices[hp])
                    if c == 0 and hp + 1 < DT:
                        # next head-pair's chunk-0 q projection as the first
                        # filler inside this hp's t-loop (not a serial block)
                        fq.insert(0, lambda h=hp + 1: q_jt(0, h))
                    accA, accB = emit_attention_hp(c, hp, ctxT_c, fq)
                    emit_normalize_hp(c, hp, ctxT_c, accA, accB)
                    for fn in fq:
                        fn()
                prev["ctxT"] = ctxT_c

            # tail: adapter/output for the last chunk (msa folded into the
            # out accumulation; adapter straight from ctxT)
            ctxT3 = prev["ctxT"]
            hT = msa_pool.tile([128, 512], BF16, tag="hT")
            nc.vector.memset(hT[64:128, :], 0.0)
            nc.vector.memset(hT[64:65, :], 1.0)
            filler_adapter(NCHUNK - 1, ctxT3, hT, psum_mm)
            for sti in range(4):
                pool, tag = (psum_sc, "s") if sti % 2 == 0 else (psum_mm, "mm")
                filler_out_st(NCHUNK - 1, sti, ctxT3, hT, pool, tag)


_NC_CACHE = None


def _get_nc():
    global _NC_CACHE
    if _NC_CACHE is None:
        _NC_CACHE = build_nc()
    return _NC_CACHE


def _prep_in_maps(y, ln_g, ln_b, qkv_w, qkv_b, msa_w, a1_w, a1_b, a2_w, a2_b):
    f = np.float32
    y = np.asarray(y, f)
    ln_g = np.asarray(ln_g, f)
    ln_b = np.asarray(ln_b, f)
    qkv_w = np.asarray(qkv_w, f)
    qkv_b = np.asarray(qkv_b, f)
    msa_w = np.asarray(msa_w, f)
    a1_w = np.asarray(a1_w, f)
    a1_b = np.asarray(a1_b, f)
    a2_w = np.asarray(a2_w, f)
    a2_b = np.asarray(a2_b, f)

    import ml_dtypes
    bf = ml_dtypes.bfloat16

    # Fold LN affine into QKV: (g*xn + b) @ W.T + c == xn @ (W*g).T + (W@b + c)
    qkv_wT = np.ascontiguousarray((qkv_w * ln_g[None, :]).T)          # [768, 2304]
    qkv_b_eff = (qkv_b + qkv_w @ ln_b).astype(f)                      # [2304]
    qkv_wT_q = np.ascontiguousarray(qkv_wT[:, :D]).astype(bf)
    qkv_wT_kv = np.ascontiguousarray(qkv_wT[:, D:]).astype(bf)
    msa_wT = np.ascontiguousarray(msa_w.T).astype(bf)                 # [768, 768]
    # fold msa into adapter layer 1: relu(msa@a1.T) == relu(ctx@(a1@msa_w).T)
    a1_wT = np.ascontiguousarray((a1_w @ msa_w).T).astype(bf)         # [768, 64]
    a2_aug = np.zeros((128, D), f)                                    # [128, 768]
    a2_aug[:R] = a2_w.T
    a2_aug[R] = a2_b
    a2_aug = a2_aug.astype(bf)

    shared = {
        "qkv_wT_q": qkv_wT_q, "qkv_wT_kv": qkv_wT_kv, "qkv_b_eff": qkv_b_eff,
        "msa_wT": msa_wT, "a1_wT": a1_wT, "a1_b": a1_b, "a2_wT_aug": a2_aug,
    }
    in_maps = [dict(shared, y=np.ascontiguousarray(y[b])) for b in range(NCORES)]
    return in_maps


def run(trace=False, **inputs):
    in_maps = _prep_in_maps(**inputs)
    nc = _get_nc()
    res = bass_utils.run_bass_kernel_spmd(
        nc, in_maps, core_ids=list(range(NCORES)), trace=trace
    )
    out = np.stack([r["out"] for r in res.results], axis=0)
    return out.astype(np.float32), res


def kernel(**inputs) -> np.ndarray:
    out, _ = run(trace=False, **inputs)
    return out
